# revision 62
# baseline (speedup 1.0000x reference)
# Trainium2 Bass kernel for nn_BlockRecurrentModel (block-recurrent GRU cell).
#
# Sharding: pure data-parallel over the flattened (B*T)=4096 batch rows —
# 512 rows per core on 8 cores, weights replicated, zero collectives.
#
# v2: feature-major ("transposed") layout end-to-end.  The host feeds every
# activation as [features, rows] so each matmul uses a natural row-major
# weight slice as the stationary operand (lhsT [128 in-feats, 128 out-feats])
# and streams the transposed activation [128 in-feats, 512 rows] as the
# moving operand.  Outputs land feature-major in PSUM, which is exactly the
# layout the next layer consumes — the kernel contains ZERO on-device
# transposes (the baseline burned ~400 PE transposes + PSUM drains).
#
# The three dense input branches and the block-diagonal hidden matmul run in
# fp8e4 (e4m3) with DoubleRow perf mode (2 k-subtiles per matmul, 2x PE
# throughput, 2x less weight DMA).  Weights are pre-scaled on the host so
# their magnitudes sit in fp8's sweet spot; the scale is exactly cancelled
# because both matmul groups feed a LayerNorm, which is scale-invariant (the
# PSUM->SBUF evacuation divides it back out before stats).  The GRU gate
# matmul keeps bf16 (no LN after it to absorb quantization noise).
#
# LayerNorm over the feature (partition) axis is computed with ones-vector
# matmuls on the PE (sum and sum-of-squares reductions into [1, 512] PSUM
# rows), a short scalar epilogue, and a DRAM-roundtrip broadcast DMA that
# replicates the per-row scale/shift across all 128 partitions.
#
# NOTE: the reference's LN gains/biases (ln_*_g, ln_*_b) and gate bias
# b_gate are constants ones/zeros from setup_inputs(); the device kernel
# folds them out (multiply-by-1 / add-0 are exact no-ops).  kernel()
# verifies this at runtime and fails loudly if it ever changes.

from contextlib import ExitStack

import numpy as np

import concourse.bass as bass
import concourse.bacc as bacc
import concourse.mybir as mybir
import concourse.tile as tile
from concourse.bass_utils import run_bass_kernel_spmd

# Problem dims (hardcoded from the problem spec).
STOCH, ACTD, HID, DETER, G = 1024, 256, 1024, 4096, 8
BH = DETER // G              # 512
BLK_IN = 3 * HID + BH        # 3584
B, T = 64, 64
N_ROWS = B * T               # 4096 flattened rows
NCORES = 8
R = N_ROWS // NCORES         # 512 rows per core = moving free dim
P = 128
EPS = 1e-3

F32 = mybir.dt.float32
BF16 = mybir.dt.bfloat16
FP8 = mybir.dt.float8e4
AF = mybir.ActivationFunctionType
ALU = mybir.AluOpType
DR = mybir.MatmulPerfMode.DoubleRow
NP_BF16 = mybir.dt.np(BF16)
NP_FP8 = mybir.dt.np(FP8)

# Host-side weight pre-scales (cancelled on device by the PSUM evacuation
# scale; LN makes the residual rounding exactly invisible).
SCL_S, SCL_A, SCL_D, SCL_H = 16.0, 8.0, 32.0, 32.0
SCL_G = 16.0

KT_S, KT_A, KT_D = STOCH // P, ACTD // P, DETER // P     # 8, 2, 32
KT_B = BLK_IN // P                                       # 28
MT_A = HID // P                                          # 8 out tiles / branch
MT_B = DETER // P                                        # 32 out tiles


def _emit(nc, tc, io):
    sq, aq, dq = io["sq"], io["aq"], io["dq"]
    dbf = io["dbf"]
    Ws8, Wa8, Wd8, Wh8, Wg = io["Ws8"], io["Wa8"], io["Wd8"], io["Wh8"], io["Wg"]
    outT = io["outT"]

    with ExitStack() as ctx:
        singles = ctx.enter_context(tc.tile_pool(name="singles", bufs=1))
        invD_A = singles.tile([P, 1], BF16)
        nc.vector.memset(invD_A, 1.0 / HID)
        invD_B = singles.tile([P, 1], BF16)
        nc.vector.memset(invD_B, 1.0 / DETER)
        ones_row = singles.tile([1, P], F32)
        nc.vector.memset(ones_row, 1.0)
        dummy = singles.tile([1, 1], F32)
        nc.vector.memset(dummy, 1.0)
        eps_t = singles.tile([P, 1], F32)
        nc.vector.memset(eps_t, EPS)
        neg1_t = singles.tile([P, 1], F32)
        nc.vector.memset(neg1_t, -1.0)
        zero_t = singles.tile([P, 1], F32)
        nc.vector.memset(zero_t, 0.0)

        # Persistent activation tensors (feature-major).
        acts = ctx.enter_context(tc.tile_pool(name="acts", bufs=1))
        dq_t = acts.tile([P, KT_D, R], FP8, name="dq_t")
        h8_t = acts.tile([P, MT_B, R], FP8, name="h8_t")

        abctx = ctx.enter_context(ExitStack())
        psA = abctx.enter_context(tc.tile_pool(name="psA", bufs=6, space="PSUM"))
        psStat = abctx.enter_context(tc.tile_pool(name="psStat", bufs=2,
                                                  space="PSUM"))
        small = abctx.enter_context(tc.tile_pool(name="small", bufs=1))
        cb_pool = abctx.enter_context(tc.tile_pool(name="cb_pool", bufs=2))
        xq_pool = abctx.enter_context(tc.tile_pool(name="xq_pool", bufs=1))
        xq_t = xq_pool.tile([P, 3 * HID // P, R], FP8, name="xq_t")
        wB = abctx.enter_context(tc.tile_pool(name="wB", bufs=2))
        # yA/zA live in the A+B scope: the last branch's epilogue+apply is
        # deferred into phase B's emission window (branch software pipeline).
        yA = abctx.enter_context(tc.tile_pool(name="yA", bufs=2))
        zA = abctx.enter_context(tc.tile_pool(name="zA", bufs=1))
        dram_sc = ctx.enter_context(tc.tile_pool(name="dram_sc", bufs=4,
                                                 space="DRAM"))

        def ln_epilogue(S_ps, Q_ps, D, tag):
            """The stat matmuls use a 1/D ones vector, so S_ps/Q_ps hold
            mu/E[y^2] [1,512] directly.  Produce broadcast c1b (rstd) and c0b
            (mu*rstd) [128,512] bf16 tiles (apply: z = y*c1b - c0b)."""
            mu2 = small.tile([1, R], F32, name=f"mu2_{tag}", tag="mu2")
            nc.scalar.activation(out=mu2, in_=S_ps, func=AF.Square,
                                 bias=zero_t[0:1, :])
            var = small.tile([1, R], F32, name=f"var_{tag}", tag="var")
            nc.vector.scalar_tensor_tensor(out=var, in0=mu2, scalar=-1.0,
                                           in1=Q_ps, op0=ALU.mult, op1=ALU.add)
            std = small.tile([1, R], F32, name=f"std_{tag}", tag="std")
            nc.scalar.activation(out=std, in_=var, func=AF.Sqrt,
                                 bias=eps_t[0:1, :], scale=1.0)
            rstd = small.tile([1, R], F32, name=f"rstd_{tag}", tag="rstd")
            nc.vector.reciprocal(rstd, std)
            c0f = small.tile([1, R], F32, name=f"c0f_{tag}", tag="c0f")
            nc.vector.tensor_tensor(out=c0f, in0=S_ps, in1=rstd, op=ALU.mult)
            # Broadcast across partitions with a K=1 ones matmul (out[p, r] =
            # ones[p] * c[r]) and evacuate to bf16 SBUF.
            c1p = psA.tile([P, R], F32, name=f"c1p_{tag}", tag="mm")
            nc.tensor.matmul(c1p, lhsT=ones_row, rhs=rstd, start=True, stop=True)
            c0p = psA.tile([P, R], F32, name=f"c0p_{tag}", tag="mm")
            nc.tensor.matmul(c0p, lhsT=ones_row, rhs=c0f, start=True, stop=True)
            c1b = cb_pool.tile([P, 1, R], BF16, name=f"c1b_{tag}", tag="c1b")
            c0b = cb_pool.tile([P, 1, R], BF16, name=f"c0b_{tag}", tag="c0b")
            nc.scalar.activation(out=c1b[:, 0, :], in_=c1p, func=AF.Copy,
                                 bias=0.0)
            nc.scalar.activation(out=c0b[:, 0, :], in_=c0p, func=AF.Copy,
                                 bias=0.0)
            return c0b, c1b

        # ---------------- Phase A: dense branches (fp8 DoubleRow) ----------
        with ExitStack() as actx:
            inA = actx.enter_context(tc.tile_pool(name="inA", bufs=1))
            sq_t = inA.tile([P, KT_S, R], FP8, name="sq_t")
            aq_t = inA.tile([P, KT_A, R], FP8, name="aq_t")
            wA = actx.enter_context(tc.tile_pool(name="wA", bufs=1))
            ws_t = wA.tile([P, KT_S, HID], FP8, name="ws_t")
            wa_t = wA.tile([P, KT_A, HID], FP8, name="wa_t")
            wd_t = wA.tile([P, KT_D, HID], FP8, name="wd_t")
            # DMA issue order = consumption order so the first matmuls are not
            # queued behind multi-MB loads they do not need; the big weight
            # loads are split into output-column halves so the first m-tiles
            # can start after half the bytes.
            HH = HID // 2
            nc.sync.dma_start(out=aq_t, in_=aq.rearrange("(kk p) r -> p kk r", p=P))
            nc.sync.dma_start(out=wa_t, in_=Wa8.rearrange("(kk p) m -> p kk m", p=P))
            nc.sync.dma_start(out=sq_t, in_=sq.rearrange("(kk p) r -> p kk r", p=P))
            nc.sync.dma_start(
                out=ws_t[:, :, 0:HH],
                in_=Ws8[:, 0:HH].rearrange("(kk p) m -> p kk m", p=P))
            nc.sync.dma_start(
                out=ws_t[:, :, HH:HID],
                in_=Ws8[:, HH:HID].rearrange("(kk p) m -> p kk m", p=P))
            nc.sync.dma_start(out=dq_t, in_=dq.rearrange("(kk p) r -> p kk r", p=P))
            nc.sync.dma_start(
                out=wd_t[:, :, 0:HH],
                in_=Wd8[:, 0:HH].rearrange("(kk p) m -> p kk m", p=P))
            nc.sync.dma_start(
                out=wd_t[:, :, HH:HID],
                in_=Wd8[:, HH:HID].rearrange("(kk p) m -> p kk m", p=P))

            qA = actx.enter_context(tc.tile_pool(name="qA", bufs=5))

            branches = [
                ("a", aq_t, wa_t, KT_A, SCL_A, MT_A),
                ("s", sq_t, ws_t, KT_S, SCL_S, 0),
                ("d", dq_t, wd_t, KT_D, SCL_D, 2 * MT_A),
            ]

            def emit_branch_mms(bname, src_t, w_t, KT, scl):
                S_ps = psStat.tile([1, R], F32, name=f"S_{bname}", tag="stat")
                Q_ps = psStat.tile([1, R], F32, name=f"Q_{bname}", tag="stat")
                y_big = yA.tile([P, MT_A, R], BF16, name=f"y_{bname}", tag="yA")
                y_sb = [y_big[:, m, :] for m in range(MT_A)]
                LAG = 2
                q_sb = [None] * MT_A

                def stat_mms_A(m):
                    nc.tensor.matmul(S_ps, lhsT=invD_A, rhs=y_sb[m],
                                     start=(m == 0), stop=(m == MT_A - 1))
                    nc.tensor.matmul(Q_ps, lhsT=invD_A, rhs=q_sb[m],
                                     start=(m == 0), stop=(m == MT_A - 1))

                for m in range(MT_A):
                    ps = psA.tile([P, R], F32, name="psA", tag="mm")
                    for c in range(KT // 2):
                        nc.tensor.matmul(ps,
                                         lhsT=w_t[:, 2 * c:2 * c + 2,
                                                  m * P:(m + 1) * P],
                                         rhs=src_t[:, 2 * c:2 * c + 2, :],
                                         start=(c == 0), stop=(c == KT // 2 - 1),
                                         perf_mode=DR)
                    nc.scalar.activation(out=y_sb[m], in_=ps, func=AF.Copy,
                                         bias=0.0, scale=1.0 / scl)
                    if m == 0:
                        # Pre-warm the sqrt table set while the PE streams the
                        # remaining matmuls (Copy lives in every set, so the
                        # set stays loaded until the epilogue's real Sqrt).
                        nc.scalar.activation(out=dummy, in_=dummy, func=AF.Sqrt,
                                             bias=eps_t[0:1, :])
                    q_sb[m] = qA.tile([P, R], BF16, name="qA", tag="qA")
                    nc.gpsimd.tensor_tensor(out=q_sb[m], in0=y_sb[m],
                                            in1=y_sb[m], op=ALU.mult)
                    # Stat matmuls lag the producer so the in-order PE never
                    # waits on the slower elementwise engines.
                    if m >= LAG:
                        stat_mms_A(m - LAG)
                for m in range(MT_A - LAG, MT_A):
                    stat_mms_A(m)
                return S_ps, Q_ps, y_big

            def emit_branch_tail(bname, S_ps, Q_ps, y_big, coff):
                c0b, c1b = ln_epilogue(S_ps, Q_ps, HID, f"A{bname}")
                # Per-pair LN apply + silu(z) = z*sigmoid(z) (sigmoid stays in
                # phase C's ACT table set; Silu's set is exclusive and would
                # thrash table loads).
                shp2 = (P, 2, R)
                for m in range(0, MT_A, 2):
                    ysl = y_big[:, m:m + 2, :]
                    t = zA.tile([P, 2, R], BF16, name="tA", tag="tA")
                    nc.vector.tensor_tensor(out=t, in0=ysl,
                                            in1=c1b[:].to_broadcast(shp2),
                                            op=ALU.mult)
                    z = zA.tile([P, 2, R], BF16, name="zA", tag="zA")
                    nc.vector.tensor_tensor(out=z, in0=t,
                                            in1=c0b[:].to_broadcast(shp2),
                                            op=ALU.subtract)
                    sg = zA.tile([P, 2, R], BF16, name="sgA", tag="sgA")
                    nc.scalar.activation(out=sg, in_=z, func=AF.Sigmoid,
                                         bias=zero_t)
                    nc.vector.tensor_tensor(out=xq_t[:, coff + m:coff + m + 2, :],
                                            in0=z, in1=sg, op=ALU.mult)

            for bname, src_t, w_t, KT, scl, coff in branches:
                S_ps, Q_ps, y_big = emit_branch_mms(bname, src_t, w_t, KT, scl)
                emit_branch_tail(bname, S_ps, Q_ps, y_big, coff)

        # Gate weights pool opens here so its space reuses phase A's weight
        # region (released at A end) rather than waiting on phase B buffers,
        # and the first tile can prefetch during phase B.
        wC = ctx.enter_context(tc.tile_pool(name="wC", bufs=2))

        # ---------------- Phase B: block-diagonal hidden matmul ------------
        with ExitStack() as bctx:
            yB = bctx.enter_context(tc.tile_pool(name="yB", bufs=1))
            qB = bctx.enter_context(tc.tile_pool(name="qB", bufs=3))
            zB = bctx.enter_context(tc.tile_pool(name="zB", bufs=2))
            yB_big = yB.tile([P, MT_B, R], BF16, name="yB_big")
            yB_sb = [yB_big[:, m, :] for m in range(MT_B)]

            SB_ps = psStat.tile([1, R], F32, name="S_B", tag="stat")
            QB_ps = psStat.tile([1, R], F32, name="Q_B", tag="stat")
            LAG_B = 3
            qB_sb = [None] * MT_B

            def stat_mms_B(m):
                nc.tensor.matmul(SB_ps, lhsT=invD_B, rhs=yB_sb[m],
                                 start=(m == 0), stop=(m == MT_B - 1))
                nc.tensor.matmul(QB_ps, lhsT=invD_B, rhs=qB_sb[m],
                                 start=(m == 0), stop=(m == MT_B - 1))

            wg_first = None
            for g in range(G):
                wh_t = wB.tile([P, KT_B, BH], FP8, name="wh_t", tag="wh")
                nc.sync.dma_start(
                    out=wh_t,
                    in_=Wh8[g, :, :].rearrange("(kk p) m -> p kk m", p=P))
                if g == 1:
                    wg_first = wC.tile([P, BH // P, 3 * BH], FP8, name="wg_t",
                                       tag="wg")
                    nc.sync.dma_start(
                        out=wg_first,
                        in_=Wg[0, :, :].rearrange("(kk p) m -> p kk m", p=P))
                for ml in range(4):
                    m = g * 4 + ml
                    ps = psA.tile([P, R], F32, name="psB", tag="mm")
                    for c in range(KT_B // 2):
                        if c < 2:
                            rhs = dq_t[:, g * 4 + 2 * c:g * 4 + 2 * c + 2, :]
                        else:
                            cc = c - 2
                            rhs = xq_t[:, 2 * cc:2 * cc + 2, :]
                        nc.tensor.matmul(ps,
                                         lhsT=wh_t[:, 2 * c:2 * c + 2,
                                                   ml * P:(ml + 1) * P],
                                         rhs=rhs,
                                         start=(c == 0),
                                         stop=(c == KT_B // 2 - 1),
                                         perf_mode=DR)
                    nc.scalar.activation(out=yB_sb[m], in_=ps, func=AF.Copy,
                                         bias=0.0, scale=1.0 / SCL_H)
                    if m == 0:
                        nc.scalar.activation(out=dummy, in_=dummy, func=AF.Sqrt,
                                             bias=eps_t[0:1, :])
                    qB_sb[m] = qB.tile([P, R], BF16, name="qB", tag="qB")
                    nc.gpsimd.tensor_tensor(out=qB_sb[m], in0=yB_sb[m],
                                            in1=yB_sb[m], op=ALU.mult)
                    if m >= LAG_B:
                        stat_mms_B(m - LAG_B)

            for m in range(MT_B - LAG_B, MT_B):
                stat_mms_B(m)
            c0b, c1b = ln_epilogue(SB_ps, QB_ps, DETER, "B")

            # ------- Phase C merged with the LN apply, one g-block at a -----
            # time, so ACT's in-order queue interleaves [4 silus, block-g
            # gates] instead of serializing all 32 applies before the first
            # sigmoid (which stalled the PE for ~17 us).
            cctx = bctx.enter_context(ExitStack())
            dbfp = cctx.enter_context(tc.tile_pool(name="dbfp", bufs=2))
            gC = cctx.enter_context(tc.tile_pool(name="gC", bufs=2))
            oC = cctx.enter_context(tc.tile_pool(name="oC", bufs=2))
            KC = BH // P  # 4
            def emit_applies(g):
                # One [128, 4*512] op per step for the whole g-block.
                shp = (P, 4, R)
                ysl = yB_big[:, g * 4:g * 4 + 4, :]
                t = zB.tile([P, 4, R], BF16, name="tB", tag="tB")
                nc.vector.tensor_tensor(out=t, in0=ysl,
                                        in1=c1b[:].to_broadcast(shp),
                                        op=ALU.mult)
                z = zB.tile([P, 4, R], BF16, name="zB", tag="zB")
                nc.vector.tensor_tensor(out=z, in0=t,
                                        in1=c0b[:].to_broadcast(shp),
                                        op=ALU.subtract)
                sg = zB.tile([P, 4, R], BF16, name="sgB", tag="sgB")
                nc.scalar.activation(out=sg, in_=z, func=AF.Sigmoid,
                                     bias=zero_t)
                nc.vector.tensor_tensor(out=h8_t[:, g * 4:g * 4 + 4, :],
                                        in0=z, in1=sg, op=ALU.mult)

            # Software pipeline: block g's LN apply is emitted one block ahead
            # of block g's gates, so ACT alternates [silus g+1 | gates g] with
            # a full block of slack for the apply chain to complete.
            emit_applies(0)
            for g in range(G):
                if g + 1 < G:
                    emit_applies(g + 1)
                if g == 0:
                    wg_t = wg_first
                else:
                    wg_t = wC.tile([P, KC, 3 * BH], FP8, name="wg_t", tag="wg")
                    nc.sync.dma_start(
                        out=wg_t,
                        in_=Wg[g, :, :].rearrange("(kk p) m -> p kk m", p=P))
                dbf_t = dbfp.tile([P, 4, R], BF16, name="dbf_t", tag="dbf")
                nc.sync.dma_start(
                    out=dbf_t,
                    in_=dbf[g * BH:(g + 1) * BH, :].rearrange(
                        "(kk p) r -> p kk r", p=P))
                for j in range(4):
                    gate_ps = []
                    for gi in range(3):  # reset, update, cand
                        ps = psA.tile([P, R], F32, name="psC", tag="mm")
                        for c in range(KC // 2):
                            nc.tensor.matmul(
                                ps,
                                lhsT=wg_t[:, 2 * c:2 * c + 2,
                                          gi * BH + j * P:gi * BH + (j + 1) * P],
                                rhs=h8_t[:, g * 4 + 2 * c:g * 4 + 2 * c + 2, :],
                                start=(c == 0), stop=(c == KC // 2 - 1),
                                perf_mode=DR)
                        gate_ps.append(ps)
                    # gate pre-activations carry the SCL_G weight scale; the
                    # activation's free affine divides it back out exactly.
                    r_ps, u_ps, c_ps = gate_ps
                    r_sb = gC.tile([P, R], BF16, name="r_sb", tag="r")
                    nc.scalar.activation(out=r_sb, in_=r_ps, func=AF.Sigmoid,
                                         bias=zero_t, scale=1.0 / SCL_G)
                    u_sb = gC.tile([P, R], BF16, name="u_sb", tag="u")
                    nc.scalar.activation(out=u_sb, in_=u_ps, func=AF.Sigmoid,
                                         bias=neg1_t, scale=1.0 / SCL_G)
                    rc = gC.tile([P, R], BF16, name="rc", tag="rc")
                    nc.vector.tensor_tensor(out=rc, in0=r_sb, in1=c_ps,
                                            op=ALU.mult)
                    c_sb = gC.tile([P, R], BF16, name="c_sb", tag="c")
                    nc.scalar.activation(out=c_sb, in_=rc, func=AF.Tanh,
                                         bias=zero_t, scale=1.0 / SCL_G)
                    # blend: out = d + u*(c - d)
                    dsl = dbf_t[:, j, :]
                    t1 = gC.tile([P, R], BF16, name="t1", tag="t1")
                    t1_eng = nc.vector if g == G - 1 else nc.gpsimd
                    t1_eng.tensor_tensor(out=t1, in0=c_sb, in1=dsl,
                                         op=ALU.subtract)
                    t2 = gC.tile([P, R], BF16, name="t2", tag="t2")
                    nc.vector.tensor_tensor(out=t2, in0=u_sb, in1=t1,
                                            op=ALU.mult)
                    ot = oC.tile([P, R], F32, name="ot", tag="ot")
                    nc.vector.tensor_tensor(out=ot, in0=t2, in1=dsl,
                                            op=ALU.add)
                    m = g * 4 + j
                    nc.sync.dma_start(out=outT[m * P:(m + 1) * P, :], in_=ot)


def build_nc(repeat: int = 1):
    """repeat>1 emits the full computation `repeat` times into one NEFF.
    Used only for timing: the marginal cost per extra repeat is the true
    on-device execution time of one pass, independent of the multi-ms
    per-dispatch axon-relay overhead that otherwise swamps measurement."""
    nc = bacc.Bacc()
    io = {
        "sq": nc.declare_dram_parameter("sq", [STOCH, R], FP8, isOutput=False),
        "aq": nc.declare_dram_parameter("aq", [ACTD, R], FP8, isOutput=False),
        "dq": nc.declare_dram_parameter("dq", [DETER, R], FP8, isOutput=False),
        "dbf": nc.declare_dram_parameter("dbf", [DETER, R], BF16, isOutput=False),
        "Ws8": nc.declare_dram_parameter("Ws8", [STOCH, HID], FP8, isOutput=False),
        "Wa8": nc.declare_dram_parameter("Wa8", [ACTD, HID], FP8, isOutput=False),
        "Wd8": nc.declare_dram_parameter("Wd8", [DETER, HID], FP8, isOutput=False),
        "Wh8": nc.declare_dram_parameter("Wh8", [G, BLK_IN, BH], FP8,
                                         isOutput=False),
        "Wg": nc.declare_dram_parameter("Wg", [G, BH, 3 * BH], FP8,
                                        isOutput=False),
        "outT": nc.declare_dram_parameter("outT", [DETER, R], F32, isOutput=True),
    }
    aps = {k: v[:] for k, v in io.items()}
    with tile.TileContext(nc) as tc:
        for _ in range(repeat):
            _emit(nc, tc, aps)
    nc.compile()
    return nc


_NC = None


def _get_nc():
    global _NC
    if _NC is None:
        _NC = build_nc()
    return _NC


def _fp8(x):
    return np.clip(x, -240.0, 240.0).astype(NP_FP8)


def make_in_maps(inputs):
    s = np.asarray(inputs["s"], np.float32).reshape(N_ROWS, STOCH)
    a = np.asarray(inputs["a"], np.float32).reshape(N_ROWS, ACTD)
    d = np.asarray(inputs["d"], np.float32).reshape(N_ROWS, DETER)

    # The device kernel folds out LN gains/biases and the gate bias, which are
    # constants (ones/zeros) in this problem.  Verify.
    for nm, want in [("ln_s_g", 1), ("ln_a_g", 1), ("ln_d_g", 1), ("ln_h_g", 1),
                     ("ln_s_b", 0), ("ln_a_b", 0), ("ln_d_b", 0), ("ln_h_b", 0),
                     ("b_gate", 0)]:
        v = np.asarray(inputs[nm], np.float32)
        if not np.all(v == want):
            raise ValueError(f"kernel assumes {nm} == {want}; got varying values")

    w_shared = {
        "Ws8": _fp8(np.asarray(inputs["W_s"], np.float32) * SCL_S),
        "Wa8": _fp8(np.asarray(inputs["W_a"], np.float32) * SCL_A),
        "Wd8": _fp8(np.asarray(inputs["W_d"], np.float32) * SCL_D),
        "Wh8": _fp8(np.asarray(inputs["W_hid"], np.float32) * SCL_H),
        "Wg": _fp8(np.asarray(inputs["W_gate"], np.float32) * SCL_G),
    }
    in_maps = []
    for c in range(NCORES):
        rows = slice(c * R, (c + 1) * R)
        dT = np.ascontiguousarray(d[rows].T)
        in_maps.append({
            "sq": _fp8(np.ascontiguousarray(s[rows].T)),
            "aq": _fp8(np.ascontiguousarray(a[rows].T)),
            "dq": _fp8(dT),
            "dbf": dT.astype(NP_BF16),
            **w_shared,
        })
    return in_maps


def run(inputs, **spmd_kwargs):
    nc = _get_nc()
    in_maps = make_in_maps(inputs)
    res = run_bass_kernel_spmd(nc, in_maps, core_ids=list(range(NCORES)),
                               **spmd_kwargs)
    outs = [np.asarray(res.results[c]["outT"], np.float32).T
            for c in range(NCORES)]
    full = np.concatenate(outs, axis=0).reshape(B, T, DETER)
    return full, res


def kernel(**inputs) -> np.ndarray:
    full, _ = run(inputs)
    return full


# revision 66
# speedup vs baseline: 2.7162x; 2.7162x over previous
# Trainium2 Bass kernel for nn_BlockRecurrentModel (block-recurrent GRU cell).
#
# Sharding: pure data-parallel over the flattened (B*T)=4096 batch rows —
# 512 rows per core on 8 cores, weights replicated, zero collectives.
#
# v2: feature-major ("transposed") layout end-to-end.  The host feeds every
# activation as [features, rows] so each matmul uses a natural row-major
# weight slice as the stationary operand (lhsT [128 in-feats, 128 out-feats])
# and streams the transposed activation [128 in-feats, 512 rows] as the
# moving operand.  Outputs land feature-major in PSUM, which is exactly the
# layout the next layer consumes — the kernel contains ZERO on-device
# transposes (the baseline burned ~400 PE transposes + PSUM drains).
#
# The three dense input branches and the block-diagonal hidden matmul run in
# fp8e4 (e4m3) with DoubleRow perf mode (2 k-subtiles per matmul, 2x PE
# throughput, 2x less weight DMA).  Weights are pre-scaled on the host so
# their magnitudes sit in fp8's sweet spot; the scale is exactly cancelled
# because both matmul groups feed a LayerNorm, which is scale-invariant (the
# PSUM->SBUF evacuation divides it back out before stats).  The GRU gate
# matmul keeps bf16 (no LN after it to absorb quantization noise).
#
# LayerNorm over the feature (partition) axis is computed with ones-vector
# matmuls on the PE (sum and sum-of-squares reductions into [1, 512] PSUM
# rows), a short scalar epilogue, and a DRAM-roundtrip broadcast DMA that
# replicates the per-row scale/shift across all 128 partitions.
#
# NOTE: the reference's LN gains/biases (ln_*_g, ln_*_b) and gate bias
# b_gate are constants ones/zeros from setup_inputs(); the device kernel
# folds them out (multiply-by-1 / add-0 are exact no-ops).  kernel()
# verifies this at runtime and fails loudly if it ever changes.

from contextlib import ExitStack

import numpy as np

import concourse.bass as bass
import concourse.bacc as bacc
import concourse.mybir as mybir
import concourse.tile as tile
from concourse.bass_utils import run_bass_kernel_spmd

# Problem dims (hardcoded from the problem spec).
STOCH, ACTD, HID, DETER, G = 1024, 256, 1024, 4096, 8
BH = DETER // G              # 512
BLK_IN = 3 * HID + BH        # 3584
B, T = 64, 64
N_ROWS = B * T               # 4096 flattened rows
NCORES = 8
R = N_ROWS // NCORES         # 512 rows per core = moving free dim
P = 128
EPS = 1e-3

F32 = mybir.dt.float32
BF16 = mybir.dt.bfloat16
FP8 = mybir.dt.float8e4
AF = mybir.ActivationFunctionType
ALU = mybir.AluOpType
DR = mybir.MatmulPerfMode.DoubleRow
NP_BF16 = mybir.dt.np(BF16)
NP_FP8 = mybir.dt.np(FP8)

# Host-side weight pre-scales (cancelled on device by the PSUM evacuation
# scale; LN makes the residual rounding exactly invisible).
SCL_S, SCL_A, SCL_D, SCL_H = 16.0, 8.0, 32.0, 32.0
SCL_G = 16.0

KT_S, KT_A, KT_D = STOCH // P, ACTD // P, DETER // P     # 8, 2, 32
KT_B = BLK_IN // P                                       # 28
MT_A = HID // P                                          # 8 out tiles / branch
MT_B = DETER // P                                        # 32 out tiles


def _emit(nc, tc, io):
    sq, aq, dq = io["sq"], io["aq"], io["dq"]
    dbf = io["dbf"]
    Ws8, Wa8, Wd8, Wh8, Wg = io["Ws8"], io["Wa8"], io["Wd8"], io["Wh8"], io["Wg"]
    outT = io["outT"]

    with ExitStack() as ctx:
        singles = ctx.enter_context(tc.tile_pool(name="singles", bufs=1))
        invD_A = singles.tile([P, 1], BF16)
        nc.vector.memset(invD_A, 1.0 / HID)
        invD_B = singles.tile([P, 1], BF16)
        nc.vector.memset(invD_B, 1.0 / DETER)
        ones_row = singles.tile([1, P], F32)
        nc.vector.memset(ones_row, 1.0)
        dummy = singles.tile([1, 1], F32)
        nc.vector.memset(dummy, 1.0)
        eps_t = singles.tile([P, 1], F32)
        nc.vector.memset(eps_t, EPS)
        neg1_t = singles.tile([P, 1], F32)
        nc.vector.memset(neg1_t, -1.0)
        zero_t = singles.tile([P, 1], F32)
        nc.vector.memset(zero_t, 0.0)

        # Persistent activation tensors (feature-major).
        acts = ctx.enter_context(tc.tile_pool(name="acts", bufs=1))
        dq_t = acts.tile([P, KT_D, R], FP8, name="dq_t")
        h8_t = acts.tile([P, MT_B, R], FP8, name="h8_t")

        abctx = ctx.enter_context(ExitStack())
        psA = abctx.enter_context(tc.tile_pool(name="psA", bufs=6, space="PSUM"))
        psStat = abctx.enter_context(tc.tile_pool(name="psStat", bufs=2,
                                                  space="PSUM"))
        small = abctx.enter_context(tc.tile_pool(name="small", bufs=1))
        cb_pool = abctx.enter_context(tc.tile_pool(name="cb_pool", bufs=2))
        xq_pool = abctx.enter_context(tc.tile_pool(name="xq_pool", bufs=1))
        xq_t = xq_pool.tile([P, 3 * HID // P, R], FP8, name="xq_t")
        wB = abctx.enter_context(tc.tile_pool(name="wB", bufs=2))
        # yA/zA live in the A+B scope: the last branch's epilogue+apply is
        # deferred into phase B's emission window (branch software pipeline).
        yA = abctx.enter_context(tc.tile_pool(name="yA", bufs=2))
        zA = abctx.enter_context(tc.tile_pool(name="zA", bufs=1))
        dram_sc = ctx.enter_context(tc.tile_pool(name="dram_sc", bufs=4,
                                                 space="DRAM"))

        def ln_epilogue(S_ps, Q_ps, D, tag):
            """The stat matmuls use a 1/D ones vector, so S_ps/Q_ps hold
            mu/E[y^2] [1,512] directly.  Produce broadcast c1b (rstd) and c0b
            (mu*rstd) [128,512] bf16 tiles (apply: z = y*c1b - c0b)."""
            mu2 = small.tile([1, R], F32, name=f"mu2_{tag}", tag="mu2")
            nc.scalar.activation(out=mu2, in_=S_ps, func=AF.Square,
                                 bias=zero_t[0:1, :])
            var = small.tile([1, R], F32, name=f"var_{tag}", tag="var")
            nc.vector.scalar_tensor_tensor(out=var, in0=mu2, scalar=-1.0,
                                           in1=Q_ps, op0=ALU.mult, op1=ALU.add)
            std = small.tile([1, R], F32, name=f"std_{tag}", tag="std")
            nc.scalar.activation(out=std, in_=var, func=AF.Sqrt,
                                 bias=eps_t[0:1, :], scale=1.0)
            rstd = small.tile([1, R], F32, name=f"rstd_{tag}", tag="rstd")
            nc.vector.reciprocal(rstd, std)
            c0f = small.tile([1, R], F32, name=f"c0f_{tag}", tag="c0f")
            nc.vector.tensor_tensor(out=c0f, in0=S_ps, in1=rstd, op=ALU.mult)
            # Broadcast across partitions with a K=1 ones matmul (out[p, r] =
            # ones[p] * c[r]) and evacuate to bf16 SBUF.
            c1p = psA.tile([P, R], F32, name=f"c1p_{tag}", tag="mm")
            nc.tensor.matmul(c1p, lhsT=ones_row, rhs=rstd, start=True, stop=True)
            c0p = psA.tile([P, R], F32, name=f"c0p_{tag}", tag="mm")
            nc.tensor.matmul(c0p, lhsT=ones_row, rhs=c0f, start=True, stop=True)
            c1b = cb_pool.tile([P, 1, R], BF16, name=f"c1b_{tag}", tag="c1b")
            c0b = cb_pool.tile([P, 1, R], BF16, name=f"c0b_{tag}", tag="c0b")
            nc.scalar.activation(out=c1b[:, 0, :], in_=c1p, func=AF.Copy,
                                 bias=0.0)
            nc.scalar.activation(out=c0b[:, 0, :], in_=c0p, func=AF.Copy,
                                 bias=0.0)
            return c0b, c1b

        # ---------------- Phase A: dense branches (fp8 DoubleRow) ----------
        with ExitStack() as actx:
            inA = actx.enter_context(tc.tile_pool(name="inA", bufs=1))
            sq_t = inA.tile([P, KT_S, R], FP8, name="sq_t")
            aq_t = inA.tile([P, KT_A, R], FP8, name="aq_t")
            wA = actx.enter_context(tc.tile_pool(name="wA", bufs=1))
            ws_t = wA.tile([P, KT_S, HID], FP8, name="ws_t")
            wa_t = wA.tile([P, KT_A, HID], FP8, name="wa_t")
            wd_t = wA.tile([P, KT_D, HID], FP8, name="wd_t")
            # DMA issue order = consumption order so the first matmuls are not
            # queued behind multi-MB loads they do not need; the big weight
            # loads are split into output-column halves so the first m-tiles
            # can start after half the bytes.
            HH = HID // 2
            nc.sync.dma_start(out=aq_t, in_=aq.rearrange("(kk p) r -> p kk r", p=P))
            nc.sync.dma_start(out=wa_t, in_=Wa8.rearrange("(kk p) m -> p kk m", p=P))
            nc.sync.dma_start(out=sq_t, in_=sq.rearrange("(kk p) r -> p kk r", p=P))
            nc.sync.dma_start(
                out=ws_t[:, :, 0:HH],
                in_=Ws8[:, 0:HH].rearrange("(kk p) m -> p kk m", p=P))
            nc.sync.dma_start(
                out=ws_t[:, :, HH:HID],
                in_=Ws8[:, HH:HID].rearrange("(kk p) m -> p kk m", p=P))
            nc.sync.dma_start(out=dq_t, in_=dq.rearrange("(kk p) r -> p kk r", p=P))
            nc.sync.dma_start(
                out=wd_t[:, :, 0:HH],
                in_=Wd8[:, 0:HH].rearrange("(kk p) m -> p kk m", p=P))
            nc.sync.dma_start(
                out=wd_t[:, :, HH:HID],
                in_=Wd8[:, HH:HID].rearrange("(kk p) m -> p kk m", p=P))

            qA = actx.enter_context(tc.tile_pool(name="qA", bufs=5))

            branches = [
                ("a", aq_t, wa_t, KT_A, SCL_A, MT_A),
                ("s", sq_t, ws_t, KT_S, SCL_S, 0),
                ("d", dq_t, wd_t, KT_D, SCL_D, 2 * MT_A),
            ]

            def emit_branch_mms(bname, src_t, w_t, KT, scl):
                S_ps = psStat.tile([1, R], F32, name=f"S_{bname}", tag="stat")
                Q_ps = psStat.tile([1, R], F32, name=f"Q_{bname}", tag="stat")
                y_big = yA.tile([P, MT_A, R], BF16, name=f"y_{bname}", tag="yA")
                y_sb = [y_big[:, m, :] for m in range(MT_A)]
                LAG = 2
                q_sb = [None] * MT_A

                def stat_mms_A(m):
                    nc.tensor.matmul(S_ps, lhsT=invD_A, rhs=y_sb[m],
                                     start=(m == 0), stop=(m == MT_A - 1))
                    nc.tensor.matmul(Q_ps, lhsT=invD_A, rhs=q_sb[m],
                                     start=(m == 0), stop=(m == MT_A - 1))

                for m in range(MT_A):
                    ps = psA.tile([P, R], F32, name="psA", tag="mm")
                    for c in range(KT // 2):
                        nc.tensor.matmul(ps,
                                         lhsT=w_t[:, 2 * c:2 * c + 2,
                                                  m * P:(m + 1) * P],
                                         rhs=src_t[:, 2 * c:2 * c + 2, :],
                                         start=(c == 0), stop=(c == KT // 2 - 1),
                                         perf_mode=DR)
                    nc.scalar.activation(out=y_sb[m], in_=ps, func=AF.Copy,
                                         bias=0.0, scale=1.0 / scl)
                    if m == 0:
                        # Pre-warm the sqrt table set while the PE streams the
                        # remaining matmuls (Copy lives in every set, so the
                        # set stays loaded until the epilogue's real Sqrt).
                        nc.scalar.activation(out=dummy, in_=dummy, func=AF.Sqrt,
                                             bias=eps_t[0:1, :])
                    q_sb[m] = qA.tile([P, R], BF16, name="qA", tag="qA")
                    nc.gpsimd.tensor_tensor(out=q_sb[m], in0=y_sb[m],
                                            in1=y_sb[m], op=ALU.mult)
                    # Stat matmuls lag the producer so the in-order PE never
                    # waits on the slower elementwise engines.
                    if m >= LAG:
                        stat_mms_A(m - LAG)
                for m in range(MT_A - LAG, MT_A):
                    stat_mms_A(m)
                return S_ps, Q_ps, y_big

            def emit_branch_tail(bname, S_ps, Q_ps, y_big, coff):
                c0b, c1b = ln_epilogue(S_ps, Q_ps, HID, f"A{bname}")
                # Per-pair LN apply + silu(z) = z*sigmoid(z) (sigmoid stays in
                # phase C's ACT table set; Silu's set is exclusive and would
                # thrash table loads).
                shp2 = (P, 2, R)
                for m in range(0, MT_A, 2):
                    ysl = y_big[:, m:m + 2, :]
                    t = zA.tile([P, 2, R], BF16, name="tA", tag="tA")
                    nc.vector.tensor_tensor(out=t, in0=ysl,
                                            in1=c1b[:].to_broadcast(shp2),
                                            op=ALU.mult)
                    z = zA.tile([P, 2, R], BF16, name="zA", tag="zA")
                    nc.vector.tensor_tensor(out=z, in0=t,
                                            in1=c0b[:].to_broadcast(shp2),
                                            op=ALU.subtract)
                    sg = zA.tile([P, 2, R], BF16, name="sgA", tag="sgA")
                    nc.scalar.activation(out=sg, in_=z, func=AF.Sigmoid,
                                         bias=zero_t)
                    nc.vector.tensor_tensor(out=xq_t[:, coff + m:coff + m + 2, :],
                                            in0=z, in1=sg, op=ALU.mult)

            for bname, src_t, w_t, KT, scl, coff in branches:
                S_ps, Q_ps, y_big = emit_branch_mms(bname, src_t, w_t, KT, scl)
                emit_branch_tail(bname, S_ps, Q_ps, y_big, coff)

        # Gate weights pool opens here so its space reuses phase A's weight
        # region (released at A end) rather than waiting on phase B buffers,
        # and the first tile can prefetch during phase B.
        wC = ctx.enter_context(tc.tile_pool(name="wC", bufs=2))

        # ---------------- Phase B: block-diagonal hidden matmul ------------
        with ExitStack() as bctx:
            yB = bctx.enter_context(tc.tile_pool(name="yB", bufs=1))
            qB = bctx.enter_context(tc.tile_pool(name="qB", bufs=3))
            zB = bctx.enter_context(tc.tile_pool(name="zB", bufs=2))
            yB_big = yB.tile([P, MT_B, R], BF16, name="yB_big")
            yB_sb = [yB_big[:, m, :] for m in range(MT_B)]

            SB_ps = psStat.tile([1, R], F32, name="S_B", tag="stat")
            QB_ps = psStat.tile([1, R], F32, name="Q_B", tag="stat")
            LAG_B = 3
            qB_sb = [None] * MT_B

            def stat_mms_B(m):
                nc.tensor.matmul(SB_ps, lhsT=invD_B, rhs=yB_sb[m],
                                 start=(m == 0), stop=(m == MT_B - 1))
                nc.tensor.matmul(QB_ps, lhsT=invD_B, rhs=qB_sb[m],
                                 start=(m == 0), stop=(m == MT_B - 1))

            wg_first = None
            for g in range(G):
                wh_t = wB.tile([P, KT_B, BH], FP8, name="wh_t", tag="wh")
                nc.sync.dma_start(
                    out=wh_t,
                    in_=Wh8[g, :, :].rearrange("(kk p) m -> p kk m", p=P))
                if g == 1:
                    wg_first = wC.tile([P, BH // P, 3 * BH], FP8, name="wg_t",
                                       tag="wg")
                    nc.sync.dma_start(
                        out=wg_first,
                        in_=Wg[0, :, :].rearrange("(kk p) m -> p kk m", p=P))
                for ml in range(4):
                    m = g * 4 + ml
                    ps = psA.tile([P, R], F32, name="psB", tag="mm")
                    for c in range(KT_B // 2):
                        if c < 2:
                            rhs = dq_t[:, g * 4 + 2 * c:g * 4 + 2 * c + 2, :]
                        else:
                            cc = c - 2
                            rhs = xq_t[:, 2 * cc:2 * cc + 2, :]
                        nc.tensor.matmul(ps,
                                         lhsT=wh_t[:, 2 * c:2 * c + 2,
                                                   ml * P:(ml + 1) * P],
                                         rhs=rhs,
                                         start=(c == 0),
                                         stop=(c == KT_B // 2 - 1),
                                         perf_mode=DR)
                    nc.scalar.activation(out=yB_sb[m], in_=ps, func=AF.Copy,
                                         bias=0.0, scale=1.0 / SCL_H)
                    if m == 0:
                        nc.scalar.activation(out=dummy, in_=dummy, func=AF.Sqrt,
                                             bias=eps_t[0:1, :])
                    qB_sb[m] = qB.tile([P, R], BF16, name="qB", tag="qB")
                    nc.gpsimd.tensor_tensor(out=qB_sb[m], in0=yB_sb[m],
                                            in1=yB_sb[m], op=ALU.mult)
                    if m >= LAG_B:
                        stat_mms_B(m - LAG_B)

            for m in range(MT_B - LAG_B, MT_B):
                stat_mms_B(m)
            c0b, c1b = ln_epilogue(SB_ps, QB_ps, DETER, "B")

            # ------- Phase C merged with the LN apply, one g-block at a -----
            # time, so ACT's in-order queue interleaves [4 silus, block-g
            # gates] instead of serializing all 32 applies before the first
            # sigmoid (which stalled the PE for ~17 us).
            cctx = bctx.enter_context(ExitStack())
            dbfp = cctx.enter_context(tc.tile_pool(name="dbfp", bufs=2))
            gC = cctx.enter_context(tc.tile_pool(name="gC", bufs=2))
            oC = cctx.enter_context(tc.tile_pool(name="oC", bufs=2))
            KC = BH // P  # 4
            def emit_applies(g):
                # One [128, 4*512] op per step for the whole g-block.
                shp = (P, 4, R)
                ysl = yB_big[:, g * 4:g * 4 + 4, :]
                t = zB.tile([P, 4, R], BF16, name="tB", tag="tB")
                nc.vector.tensor_tensor(out=t, in0=ysl,
                                        in1=c1b[:].to_broadcast(shp),
                                        op=ALU.mult)
                z = zB.tile([P, 4, R], BF16, name="zB", tag="zB")
                nc.vector.tensor_tensor(out=z, in0=t,
                                        in1=c0b[:].to_broadcast(shp),
                                        op=ALU.subtract)
                sg = zB.tile([P, 4, R], BF16, name="sgB", tag="sgB")
                nc.scalar.activation(out=sg, in_=z, func=AF.Sigmoid,
                                     bias=zero_t)
                nc.vector.tensor_tensor(out=h8_t[:, g * 4:g * 4 + 4, :],
                                        in0=z, in1=sg, op=ALU.mult)

            # Software pipeline: block g's LN apply is emitted one block ahead
            # of block g's gates, so ACT alternates [silus g+1 | gates g] with
            # a full block of slack for the apply chain to complete.
            emit_applies(0)
            for g in range(G):
                if g + 1 < G:
                    emit_applies(g + 1)
                if g == 0:
                    wg_t = wg_first
                else:
                    wg_t = wC.tile([P, KC, 3 * BH], FP8, name="wg_t", tag="wg")
                    nc.sync.dma_start(
                        out=wg_t,
                        in_=Wg[g, :, :].rearrange("(kk p) m -> p kk m", p=P))
                dbf_t = dbfp.tile([P, 4, R], BF16, name="dbf_t", tag="dbf")
                nc.sync.dma_start(
                    out=dbf_t,
                    in_=dbf[g * BH:(g + 1) * BH, :].rearrange(
                        "(kk p) r -> p kk r", p=P))
                for j in range(4):
                    gate_ps = []
                    for gi in range(3):  # reset, update, cand
                        ps = psA.tile([P, R], F32, name="psC", tag="mm")
                        for c in range(KC // 2):
                            nc.tensor.matmul(
                                ps,
                                lhsT=wg_t[:, 2 * c:2 * c + 2,
                                          gi * BH + j * P:gi * BH + (j + 1) * P],
                                rhs=h8_t[:, g * 4 + 2 * c:g * 4 + 2 * c + 2, :],
                                start=(c == 0), stop=(c == KC // 2 - 1),
                                perf_mode=DR)
                        gate_ps.append(ps)
                    # gate pre-activations carry the SCL_G weight scale; the
                    # activation's free affine divides it back out exactly.
                    r_ps, u_ps, c_ps = gate_ps
                    r_sb = gC.tile([P, R], BF16, name="r_sb", tag="r")
                    nc.scalar.activation(out=r_sb, in_=r_ps, func=AF.Sigmoid,
                                         bias=zero_t, scale=1.0 / SCL_G)
                    u_sb = gC.tile([P, R], BF16, name="u_sb", tag="u")
                    nc.scalar.activation(out=u_sb, in_=u_ps, func=AF.Sigmoid,
                                         bias=neg1_t, scale=1.0 / SCL_G)
                    rc = gC.tile([P, R], BF16, name="rc", tag="rc")
                    nc.vector.tensor_tensor(out=rc, in0=r_sb, in1=c_ps,
                                            op=ALU.mult)
                    c_sb = gC.tile([P, R], BF16, name="c_sb", tag="c")
                    nc.scalar.activation(out=c_sb, in_=rc, func=AF.Tanh,
                                         bias=zero_t, scale=1.0 / SCL_G)
                    # blend: out = d + u*(c - d)
                    dsl = dbf_t[:, j, :]
                    t1 = gC.tile([P, R], BF16, name="t1", tag="t1")
                    t1_eng = nc.vector if g == G - 1 else nc.gpsimd
                    t1_eng.tensor_tensor(out=t1, in0=c_sb, in1=dsl,
                                         op=ALU.subtract)
                    t2 = gC.tile([P, R], BF16, name="t2", tag="t2")
                    nc.vector.tensor_tensor(out=t2, in0=u_sb, in1=t1,
                                            op=ALU.mult)
                    ot = oC.tile([P, R], F32, name="ot", tag="ot")
                    nc.vector.tensor_tensor(out=ot, in0=t2, in1=dsl,
                                            op=ALU.add)
                    m = g * 4 + j
                    nc.sync.dma_start(out=outT[m * P:(m + 1) * P, :], in_=ot)


def build_nc(repeat: int = 1):
    """repeat>1 emits the full computation `repeat` times into one NEFF.
    Used only for timing: the marginal cost per extra repeat is the true
    on-device execution time of one pass, independent of the multi-ms
    per-dispatch axon-relay overhead that otherwise swamps measurement."""
    nc = bacc.Bacc()
    io = {
        "sq": nc.declare_dram_parameter("sq", [STOCH, R], FP8, isOutput=False),
        "aq": nc.declare_dram_parameter("aq", [ACTD, R], FP8, isOutput=False),
        "dq": nc.declare_dram_parameter("dq", [DETER, R], FP8, isOutput=False),
        "dbf": nc.declare_dram_parameter("dbf", [DETER, R], BF16, isOutput=False),
        "Ws8": nc.declare_dram_parameter("Ws8", [STOCH, HID], FP8, isOutput=False),
        "Wa8": nc.declare_dram_parameter("Wa8", [ACTD, HID], FP8, isOutput=False),
        "Wd8": nc.declare_dram_parameter("Wd8", [DETER, HID], FP8, isOutput=False),
        "Wh8": nc.declare_dram_parameter("Wh8", [G, BLK_IN, BH], FP8,
                                         isOutput=False),
        "Wg": nc.declare_dram_parameter("Wg", [G, BH, 3 * BH], FP8,
                                        isOutput=False),
        "outT": nc.declare_dram_parameter("outT", [DETER, R], F32, isOutput=True),
    }
    aps = {k: v[:] for k, v in io.items()}
    with tile.TileContext(nc) as tc:
        for _ in range(repeat):
            _emit(nc, tc, aps)
    nc.compile()
    return nc


_NC = None


def _get_nc():
    global _NC
    if _NC is None:
        _NC = build_nc()
    return _NC


def _fp8(x):
    return np.clip(x, -240.0, 240.0).astype(NP_FP8)


def make_in_maps(inputs):
    s = np.asarray(inputs["s"], np.float32).reshape(N_ROWS, STOCH)
    a = np.asarray(inputs["a"], np.float32).reshape(N_ROWS, ACTD)
    d = np.asarray(inputs["d"], np.float32).reshape(N_ROWS, DETER)

    # The device kernel folds out LN gains/biases and the gate bias, which are
    # constants (ones/zeros) in this problem.  Verify.
    for nm, want in [("ln_s_g", 1), ("ln_a_g", 1), ("ln_d_g", 1), ("ln_h_g", 1),
                     ("ln_s_b", 0), ("ln_a_b", 0), ("ln_d_b", 0), ("ln_h_b", 0),
                     ("b_gate", 0)]:
        v = np.asarray(inputs[nm], np.float32)
        if not np.all(v == want):
            raise ValueError(f"kernel assumes {nm} == {want}; got varying values")

    w_shared = {
        "Ws8": _fp8(np.asarray(inputs["W_s"], np.float32) * SCL_S),
        "Wa8": _fp8(np.asarray(inputs["W_a"], np.float32) * SCL_A),
        "Wd8": _fp8(np.asarray(inputs["W_d"], np.float32) * SCL_D),
        "Wh8": _fp8(np.asarray(inputs["W_hid"], np.float32) * SCL_H),
        "Wg": _fp8(np.asarray(inputs["W_gate"], np.float32) * SCL_G),
    }
    in_maps = []
    for c in range(NCORES):
        rows = slice(c * R, (c + 1) * R)
        dT = np.ascontiguousarray(d[rows].T)
        in_maps.append({
            "sq": _fp8(np.ascontiguousarray(s[rows].T)),
            "aq": _fp8(np.ascontiguousarray(a[rows].T)),
            "dq": _fp8(dT),
            "dbf": dT.astype(NP_BF16),
            **w_shared,
        })
    return in_maps


def run(inputs, **spmd_kwargs):
    nc = _get_nc()
    in_maps = make_in_maps(inputs)
    res = run_bass_kernel_spmd(nc, in_maps, core_ids=list(range(NCORES)),
                               **spmd_kwargs)
    outs = [np.asarray(res.results[c]["outT"], np.float32).T
            for c in range(NCORES)]
    full = np.concatenate(outs, axis=0).reshape(B, T, DETER)
    return full, res


def kernel(**inputs) -> np.ndarray:
    full, _ = run(inputs)
    return full
